# revision 31
# baseline (speedup 1.0000x reference)
"""Trainium2 Bass kernel for nn_CustomMultiHeadAttention (B2 T2048 D1024 H16).

Sharding: 8 cores = 2 batches x 4 head-groups (4 heads/core, tensor-parallel
columns for Wq/Wk/Wv, rows for Wo; host sums the 4 row-parallel partials).

Math: F_ij = bs*(fj-fi)/(fi*fj+eps) ~= bs*(1/fi - 1/fj).  The 1/fi row term
is softmax-invariant, so F collapses to a per-column logit bias
c_j = -bs*scale/f_j.

Layout trick: compute S TRANSPOSED (S^T[j,i] = K_j . Q_i).  Then:
  * c_j is per-PARTITION -> folds into the ACT exp bias (exact fp32, free,
    together with a fixed safe shift A=30 >= max |scale*QK| by
    Cauchy-Schwarz on randn inputs);
  * exp output P~^T is already in the [j, i] layout the PV matmul needs --
    zero DMA transposes of the 32MB probability matrix;
  * softmax denominators come free as a 65th all-ones V column in PV
    (row 64 of each head's O^T accumulator);
  * normalization: small per-head O^T -> O transpose (PE), divide by the
    denom column (DVE, per-partition), transpose back -> paired out-proj.

Pipeline: proj Q,K -> S^T/exp for the first head-pair starts on ACT while
the V projection + V transpose run on PE (separate PSUM pools); PV trails
the exp stream using per-jc probability tiles (deep slot pool, no WAR
stalls); normalization dance + out-proj interleaved at the tail.
"""

from contextlib import ExitStack

import numpy as np
import ml_dtypes

import concourse.bass as bass
import concourse.mybir as mybir
import concourse.tile as tile
from concourse import bacc
from concourse.bass_utils import run_bass_kernel_spmd
from concourse.masks import make_identity

AF = mybir.ActivationFunctionType
ALU = mybir.AluOpType
F32 = mybir.dt.float32
BF16 = mybir.dt.bfloat16
X = mybir.AxisListType.X

B, T, D = 2, 2048, 1024
H, DH = 16, 64
H_LOC = 4
C_LOC = H_LOC * DH          # 256
N_CORES = 8
SCALE = DH ** -0.5
P = 128
ICH, JCH, KCH = T // P, T // P, D // P   # 16, 16, 8
SL2 = 2                     # i-slices
IW = T // SL2               # 1024 i-columns per slice
VW = DH + 1                 # 65: V columns + ones (denominator)
A_SHIFT = 30.0              # fixed exp upper bound on |scale*QK|


def _build_program():
    nc = bacc.Bacc("TRN2", target_bir_lowering=False, debug=False,
                   num_devices=N_CORES)

    xq_d = nc.dram_tensor("xq", [D, T], BF16, kind="ExternalInput").ap()
    xk_d = nc.dram_tensor("xk", [D, T], BF16, kind="ExternalInput").ap()
    xv_d = nc.dram_tensor("xv", [D, T], BF16, kind="ExternalInput").ap()
    wq_d = nc.dram_tensor("wq", [D, C_LOC], BF16, kind="ExternalInput").ap()
    wk_d = nc.dram_tensor("wk", [D, C_LOC], BF16, kind="ExternalInput").ap()
    wv_d = nc.dram_tensor("wv", [D, C_LOC], BF16, kind="ExternalInput").ap()
    wo_d = nc.dram_tensor("wo", [C_LOC, D], BF16, kind="ExternalInput").ap()
    cb_d = nc.dram_tensor("cb", [P, JCH], F32, kind="ExternalInput").ap()
    out_d = nc.dram_tensor("out", [T, D], BF16, kind="ExternalOutput").ap()

    with tile.TileContext(nc) as tc, ExitStack() as ctx:
        const = ctx.enter_context(tc.tile_pool(name="const", bufs=1))
        wpool = ctx.enter_context(tc.tile_pool(name="w", bufs=1))
        qkv = ctx.enter_context(tc.tile_pool(name="qkv", bufs=1))
        xpool = ctx.enter_context(tc.tile_pool(name="x", bufs=4))
        rvpool = ctx.enter_context(tc.tile_pool(name="rv", bufs=4))
        ptpool = ctx.enter_context(tc.tile_pool(name="pt", bufs=20))
        onpool = ctx.enter_context(tc.tile_pool(name="on", bufs=3))
        opool = ctx.enter_context(tc.tile_pool(name="o", bufs=2))
        psum = ctx.enter_context(tc.tile_pool(name="ps", bufs=2, space="PSUM"))
        pvps = ctx.enter_context(tc.tile_pool(name="pv", bufs=2, space="PSUM"))

        identf = const.tile([P, P], F32)
        make_identity(nc, identf)
        identb = const.tile([P, P], BF16)
        make_identity(nc, identb)
        cb_s = const.tile([P, JCH], F32)
        nc.gpsimd.dma_start(cb_s[:], cb_d)

        wq_s = wpool.tile([P, KCH, C_LOC], BF16, tag="wq")
        nc.sync.dma_start(wq_s[:], wq_d.rearrange("(kc p) c -> p kc c", p=P))
        wk_s = wpool.tile([P, KCH, C_LOC], BF16, tag="wk")
        nc.scalar.dma_start(wk_s[:], wk_d.rearrange("(kc p) c -> p kc c", p=P))
        wv_s = wpool.tile([P, KCH, C_LOC], BF16, tag="wv")
        nc.gpsimd.dma_start(wv_s[:], wv_d.rearrange("(kc p) c -> p kc c", p=P))
        wo_s = wpool.tile([P, 2, D], BF16, tag="wo")
        nc.scalar.dma_start(wo_s[:], wo_d.rearrange("(cc p) o -> p cc o", p=P))

        # ---- projections: dst[c % 128, pair, t] = (W.T x^T)  bf16 ----
        # x loads spread across the three DGE paths (sync/scalar/gpsimd)
        # so the streams drain in parallel.
        qt_s = qkv.tile([P, 2, T], BF16, tag="qt")
        kt_s = qkv.tile([P, 2, T], BF16, tag="kt")
        # vt (projection staging) and ot65 (PV output) have disjoint
        # lifetimes -> share one 32KB slot.
        vt_s = qkv.tile([P, 2, T], F32, tag="big")

        def _proj_th(x_d, w_s, dst, dma_eng, xtag, ppool, copy_fn, th):
            t0 = th * 1024
            pstiles = [ppool.tile([P, 1024], F32,
                                  tag="ps" if ppool is psum else "pv",
                                  name=f"pj{xtag}{th}{pi}")
                       for pi in range(2)]
            for kc in range(KCH):
                xt = xpool.tile([P, 1024], BF16, tag=xtag)
                dma_eng.dma_start(
                    xt[:], x_d[kc * P:(kc + 1) * P, t0:t0 + 1024])
                for pair in range(2):
                    lhsT = w_s[:, kc, pair * P:(pair + 1) * P]
                    for nb in range(2):
                        nc.tensor.matmul(
                            pstiles[pair][:, nb * 512:(nb + 1) * 512],
                            lhsT, xt[:, nb * 512:(nb + 1) * 512],
                            start=(kc == 0), stop=(kc == KCH - 1))
            for pair in range(2):
                copy_fn(dst[:, pair, t0:t0 + 1024], pstiles[pair][:])

        def _proj(x_d, w_s, dst, dma_eng, xtag, ppool, copy_fn):
            for th in range(2):                      # halves of T
                _proj_th(x_d, w_s, dst, dma_eng, xtag, ppool, copy_fn, th)

        # ---- V^T -> V[t % 128, tc, h*65 + c] bf16 via PE transpose ----
        # 65-column head stride; column h*65+64 stays 1.0 (denominator).
        v_s = qkv.tile([P, ICH, H_LOC * VW], BF16, tag="v")
        nc.any.memset(v_s[:], 1.0)

        def _v_transpose():
            for tc_i in range(ICH):
                for pair in range(2):
                    tp = pvps.tile([P, IW], F32, tag="pv",
                                   name=f"tp{tc_i}_{pair}")
                    nc.tensor.transpose(
                        tp[:, 0:P], vt_s[:, pair, tc_i * P:(tc_i + 1) * P],
                        identf[:])
                    for hh in range(2):
                        h = 2 * pair + hh
                        nc.vector.tensor_copy(
                            v_s[:, tc_i, h * VW:h * VW + DH],
                            tp[:, hh * DH:(hh + 1) * DH])

        otn = qkv.tile([P, 2, T], BF16, tag="otn")
        ot65 = qkv.tile([VW, SL2 * H_LOC, IW], F32, tag="big",
                        name="ot65")

        # ---- main loop pieces ----
        pt_tiles = {}

        def _sexp_jc(sl2, pair, jc):
            """S^T matmuls + exp for one (i-slice, head-pair, j-chunk)."""
            i0 = sl2 * IW
            sp = [psum.tile([P, IW], F32, tag="ps",
                            name=f"sp{sl2}{pair}{jc}{hh}")
                  for hh in range(2)]
            for ib in range(2):
                for hh in range(2):
                    nc.tensor.matmul(
                        sp[hh][:, ib * 512:(ib + 1) * 512],
                        kt_s[hh * 64:(hh + 1) * 64, pair,
                             jc * P:(jc + 1) * P],
                        qt_s[hh * 64:(hh + 1) * 64, pair,
                             i0 + ib * 512:i0 + (ib + 1) * 512],
                        start=True, stop=True,
                        tile_position=(64 * hh, 0))
            for hh in range(2):
                ptt = ptpool.tile([P, IW], BF16, tag=f"pt{hh}",
                                  name=f"pt{hh}_{sl2}{pair}{jc}")
                pt_tiles[(sl2, pair, jc, hh)] = ptt
                nc.scalar.activation(
                    ptt[:], sp[hh][:],
                    AF.Exp, bias=cb_s[:, jc:jc + 1], scale=SCALE)

        def _pv_jc(sl2, pair, jc, ov):
            """PV chunk with 65th ones-column (denominator in row 64)."""
            for hh in range(2):
                h = 2 * pair + hh
                ptt = pt_tiles[(sl2, pair, jc, hh)]
                for ib in range(2):
                    nc.tensor.matmul(
                        ov[hh][:, ib * 512:(ib + 1) * 512],
                        v_s[:, jc, h * VW:(h + 1) * VW],
                        ptt[:, ib * 512:(ib + 1) * 512],
                        start=(jc == 0), stop=(jc == JCH - 1))

        def _ov_flush(sl2, pair, ov):
            for hh in range(2):
                k = sl2 * H_LOC + 2 * pair + hh
                nc.vector.tensor_copy(ot65[:, k, :], ov[hh][:])

        def _ov_alloc(sl2, pair, pool):
            return [pool.tile([VW, IW], F32,
                              tag="pv" if pool is pvps else "ps",
                              name=f"ov{sl2}{pair}{hh}") for hh in range(2)]

        # ---- emission: proj Q,K -> first S/exp overlaps proj V; each
        # later pair's exp stream interleaves the previous pair's PV at
        # j-chunk granularity (constant pt-slot pressure, no ACT stalls)
        _proj(xq_d, wq_s, qt_s, nc.sync, "xq", psum, nc.scalar.copy)
        _proj(xk_d, wk_s, kt_s, nc.scalar, "xk", psum, nc.scalar.copy)
        for jc in range(JCH):
            _sexp_jc(0, 0, jc)
        _proj(xv_d, wv_s, vt_s, nc.gpsimd, "xv", pvps, nc.vector.tensor_copy)
        _v_transpose()
        steps = [(0, 1), (1, 0), (1, 1)]
        prev = (0, 0)
        ov = _ov_alloc(*prev, pvps)
        for cur in steps:
            for jc in range(JCH):
                _sexp_jc(cur[0], cur[1], jc)
                _pv_jc(prev[0], prev[1], jc, ov)
            _ov_flush(*prev, ov)
            prev = cur
            ov = _ov_alloc(*prev, pvps)
        for jc in range(JCH):
            _pv_jc(prev[0], prev[1], jc, ov)
        _ov_flush(*prev, ov)

        # ---- normalize + repack: O^T[65,i] -> O -> /denom -> O^T paired,
        # interleaved with the out projection per i-block ----
        def _dance(sl2, pair, ib):
            tp = pvps.tile([P, IW], F32, tag="pv",
                           name=f"dtp{sl2}{pair}{ib}")
            onorm = onpool.tile([P, P], BF16, tag="on")
            for hh in range(2):
                k = sl2 * H_LOC + 2 * pair + hh
                nc.tensor.transpose(
                    tp[:, hh * VW:(hh + 1) * VW],
                    ot65[:, k, ib * P:(ib + 1) * P],
                    identf[0:VW, 0:VW])
            for hh in range(2):
                rv = rvpool.tile([P, 1], F32, tag="rv",
                                 name=f"rv{sl2}{pair}{ib}{hh}")
                nc.vector.reciprocal(
                    rv[:], tp[:, hh * VW + DH:hh * VW + VW])
                nc.vector.tensor_scalar_mul(
                    onorm[:, hh * DH:(hh + 1) * DH],
                    tp[:, hh * VW:hh * VW + DH], rv[:])
            tpo = psum.tile([P, 1024], BF16, tag="ps",
                            name=f"dto{sl2}{pair}{ib}")
            nc.tensor.transpose(tpo[:, 0:P], onorm[:], identb[:])
            nc.scalar.copy(
                otn[:, pair, sl2 * IW + ib * P:sl2 * IW + (ib + 1) * P],
                tpo[:, 0:P])

        def _outproj(tb):
            ops = psum.tile([P, 1024], F32, tag="ps", name=f"op{tb}")
            for cc in range(2):
                lhsT = otn[:, cc, tb * P:(tb + 1) * P]
                for nb in range(2):
                    nc.tensor.matmul(
                        ops[:, nb * 512:(nb + 1) * 512], lhsT,
                        wo_s[:, cc, nb * 512:(nb + 1) * 512],
                        start=(cc == 0), stop=(cc == 1))
            ostage = opool.tile([P, D], BF16, tag="ostage")
            nc.scalar.copy(ostage[:], ops[:])
            nc.sync.dma_start(out_d[tb * P:(tb + 1) * P, :], ostage[:])

        for sl2 in range(SL2):
            for ib in range(IW // P):
                _dance(sl2, 0, ib)
                _dance(sl2, 1, ib)
                _outproj(sl2 * (IW // P) + ib)

    nc.compile()
    return nc


_last_results = None


def _host_cb(frac: np.ndarray, bs: float):
    """Per-j exp bias cb[p, jc] = scale*(c_j - maxc) - A, j = jc*128 + p,
    with c_j = -bs/f_j (raw logit units)."""
    cbs = []
    for b in range(B):
        f = np.maximum(frac[b].astype(np.float64), 1e-7)
        c = -bs / f
        cb = SCALE * (c - c.max()) - A_SHIFT
        cbs.append(np.ascontiguousarray(
            cb.reshape(JCH, P).T.astype(np.float32)))
    return cbs


def _prepare(inputs):
    """Build the program and per-core input maps from full inputs."""
    inp = {k: np.asarray(v) for k, v in inputs.items()}
    query, key, value = inp["query"], inp["key"], inp["value"]
    frac = inp["frac"]
    Wq, Wk, Wv, Wo = inp["Wq"], inp["Wk"], inp["Wv"], inp["Wo"]
    attn_bias = inp["attn_bias"]

    bs = float(np.sum(attn_bias.astype(np.float64)))
    cbs = _host_cb(np.asarray(frac, np.float32), bs)

    nc = _build_program()

    in_maps = []
    for c in range(N_CORES):
        b, g = c // H_LOC, c % H_LOC
        sl = slice(g * C_LOC, (g + 1) * C_LOC)
        in_maps.append({
            "xq": np.ascontiguousarray(query[b].T).astype(ml_dtypes.bfloat16),
            "xk": np.ascontiguousarray(key[b].T).astype(ml_dtypes.bfloat16),
            "xv": np.ascontiguousarray(value[b].T).astype(ml_dtypes.bfloat16),
            "wq": np.ascontiguousarray(Wq[sl, :].T).astype(ml_dtypes.bfloat16),
            "wk": np.ascontiguousarray(Wk[sl, :].T).astype(ml_dtypes.bfloat16),
            "wv": np.ascontiguousarray(Wv[sl, :].T).astype(ml_dtypes.bfloat16),
            "wo": np.ascontiguousarray(Wo[:, sl].T).astype(ml_dtypes.bfloat16),
            "cb": cbs[b],
        })
    return nc, in_maps


def kernel(**inputs) -> np.ndarray:
    nc, in_maps = _prepare(inputs)

    res = run_bass_kernel_spmd(nc, in_maps, list(range(N_CORES)))
    global _last_results
    _last_results = res

    out = np.zeros((B, T, D), dtype=np.float32)
    for c in range(N_CORES):
        out[c // H_LOC] += np.asarray(res.results[c]["out"]).astype(np.float32)
    out += np.asarray(inputs["bo"], dtype=np.float32)[None, None, :]
    return out


# revision 33
# speedup vs baseline: 1.1702x; 1.1702x over previous
"""Trainium2 Bass kernel for nn_CustomMultiHeadAttention (B2 T2048 D1024 H16).

Sharding: 8 cores = 2 batches x 4 head-groups (4 heads/core, tensor-parallel
columns for Wq/Wk/Wv, rows for Wo; host sums the 4 row-parallel partials).

Math: F_ij = bs*(fj-fi)/(fi*fj+eps) ~= bs*(1/fi - 1/fj).  The 1/fi row term
is softmax-invariant, so F collapses to a per-column logit bias
c_j = -bs*scale/f_j.

Layout trick: compute S TRANSPOSED (S^T[j,i] = K_j . Q_i).  Then:
  * c_j is per-PARTITION -> folds into the ACT exp bias (exact fp32, free,
    together with a fixed safe shift A=30 >= max |scale*QK| by
    Cauchy-Schwarz on randn inputs);
  * exp output P~^T is already in the [j, i] layout the PV matmul needs --
    zero DMA transposes of the 32MB probability matrix;
  * softmax denominators come free as a 65th all-ones V column in PV
    (row 64 of each head's O^T accumulator);
  * normalization: small per-head O^T -> O transpose (PE), divide by the
    denom column (DVE, per-partition), transpose back -> paired out-proj.

Pipeline: proj Q,K -> S^T/exp for the first head-pair starts on ACT while
the V projection + V transpose run on PE (separate PSUM pools); PV trails
the exp stream using per-jc probability tiles (deep slot pool, no WAR
stalls); normalization dance + out-proj interleaved at the tail.
"""

from contextlib import ExitStack

import numpy as np
import ml_dtypes

import concourse.bass as bass
import concourse.mybir as mybir
import concourse.tile as tile
from concourse import bacc
from concourse.bass_utils import run_bass_kernel_spmd
from concourse.masks import make_identity

AF = mybir.ActivationFunctionType
ALU = mybir.AluOpType
F32 = mybir.dt.float32
BF16 = mybir.dt.bfloat16
X = mybir.AxisListType.X

B, T, D = 2, 2048, 1024
H, DH = 16, 64
H_LOC = 4
C_LOC = H_LOC * DH          # 256
N_CORES = 8
SCALE = DH ** -0.5
P = 128
ICH, JCH, KCH = T // P, T // P, D // P   # 16, 16, 8
SL2 = 2                     # i-slices
IW = T // SL2               # 1024 i-columns per slice
VW = DH + 1                 # 65: V columns + ones (denominator)
A_SHIFT = 30.0              # fixed exp upper bound on |scale*QK|


def _build_program():
    nc = bacc.Bacc("TRN2", target_bir_lowering=False, debug=False,
                   num_devices=N_CORES)

    xq_d = nc.dram_tensor("xq", [D, T], BF16, kind="ExternalInput").ap()
    xk_d = nc.dram_tensor("xk", [D, T], BF16, kind="ExternalInput").ap()
    xv_d = nc.dram_tensor("xv", [D, T], BF16, kind="ExternalInput").ap()
    wq_d = nc.dram_tensor("wq", [D, C_LOC], BF16, kind="ExternalInput").ap()
    wk_d = nc.dram_tensor("wk", [D, C_LOC], BF16, kind="ExternalInput").ap()
    wv_d = nc.dram_tensor("wv", [D, C_LOC], BF16, kind="ExternalInput").ap()
    wo_d = nc.dram_tensor("wo", [C_LOC, D], BF16, kind="ExternalInput").ap()
    cb_d = nc.dram_tensor("cb", [P, JCH], F32, kind="ExternalInput").ap()
    out_d = nc.dram_tensor("out", [T, D], BF16, kind="ExternalOutput").ap()

    with tile.TileContext(nc) as tc, ExitStack() as ctx:
        const = ctx.enter_context(tc.tile_pool(name="const", bufs=1))
        wpool = ctx.enter_context(tc.tile_pool(name="w", bufs=1))
        qkv = ctx.enter_context(tc.tile_pool(name="qkv", bufs=1))
        xpool = ctx.enter_context(tc.tile_pool(name="x", bufs=4))
        rvpool = ctx.enter_context(tc.tile_pool(name="rv", bufs=4))
        ptpool = ctx.enter_context(tc.tile_pool(name="pt", bufs=20))
        onpool = ctx.enter_context(tc.tile_pool(name="on", bufs=3))
        opool = ctx.enter_context(tc.tile_pool(name="o", bufs=2))
        psum = ctx.enter_context(tc.tile_pool(name="ps", bufs=2, space="PSUM"))
        pvps = ctx.enter_context(tc.tile_pool(name="pv", bufs=2, space="PSUM"))

        identf = const.tile([P, P], F32)
        make_identity(nc, identf)
        identb = const.tile([P, P], BF16)
        make_identity(nc, identb)
        cb_s = const.tile([P, JCH], F32)
        nc.gpsimd.dma_start(cb_s[:], cb_d)

        wq_s = wpool.tile([P, KCH, C_LOC], BF16, tag="wq")
        nc.scalar.dma_start(wq_s[:], wq_d.rearrange("(kc p) c -> p kc c", p=P))
        wk_s = wpool.tile([P, KCH, C_LOC], BF16, tag="wk")
        nc.scalar.dma_start(wk_s[:], wk_d.rearrange("(kc p) c -> p kc c", p=P))
        wv_s = wpool.tile([P, KCH, C_LOC], BF16, tag="wv")
        nc.gpsimd.dma_start(wv_s[:], wv_d.rearrange("(kc p) c -> p kc c", p=P))
        wo_s = wpool.tile([P, 2, D], BF16, tag="wo")
        nc.scalar.dma_start(wo_s[:], wo_d.rearrange("(cc p) o -> p cc o", p=P))

        # ---- projections: dst[c % 128, pair, t] = (W.T x^T)  bf16 ----
        # x loads spread across the three DGE paths (sync/scalar/gpsimd)
        # so the streams drain in parallel.
        qt_s = qkv.tile([P, 2, T], BF16, tag="qt")
        kt_s = qkv.tile([P, 2, T], BF16, tag="kt")
        # vt (projection staging) and ot65 (PV output) have disjoint
        # lifetimes -> share one 32KB slot.
        vt_s = qkv.tile([P, 2, T], F32, tag="big")

        def _proj_th(x_d, w_s, dst, dma_eng, xtag, ppool, copy_fn, th):
            t0 = th * 1024
            pstiles = [ppool.tile([P, 1024], F32,
                                  tag="ps" if ppool is psum else "pv",
                                  name=f"pj{xtag}{th}{pi}")
                       for pi in range(2)]
            for kc in range(KCH):
                xt = xpool.tile([P, 1024], BF16, tag=xtag)
                dma_eng.dma_start(
                    xt[:], x_d[kc * P:(kc + 1) * P, t0:t0 + 1024])
                for pair in range(2):
                    lhsT = w_s[:, kc, pair * P:(pair + 1) * P]
                    for nb in range(2):
                        nc.tensor.matmul(
                            pstiles[pair][:, nb * 512:(nb + 1) * 512],
                            lhsT, xt[:, nb * 512:(nb + 1) * 512],
                            start=(kc == 0), stop=(kc == KCH - 1))
            for pair in range(2):
                copy_fn(dst[:, pair, t0:t0 + 1024], pstiles[pair][:])

        def _proj(x_d, w_s, dst, dma_eng, xtag, ppool, copy_fn):
            for th in range(2):                      # halves of T
                _proj_th(x_d, w_s, dst, dma_eng, xtag, ppool, copy_fn, th)

        # ---- V^T -> V[t % 128, tc, h*65 + c] bf16 via PE transpose ----
        # 65-column head stride; column h*65+64 stays 1.0 (denominator).
        v_s = qkv.tile([P, ICH, H_LOC * VW], BF16, tag="v")
        nc.any.memset(v_s[:], 1.0)

        def _v_transpose():
            for tc_i in range(ICH):
                for pair in range(2):
                    tp = pvps.tile([P, IW], F32, tag="pv",
                                   name=f"tp{tc_i}_{pair}")
                    nc.tensor.transpose(
                        tp[:, 0:P], vt_s[:, pair, tc_i * P:(tc_i + 1) * P],
                        identf[:])
                    for hh in range(2):
                        h = 2 * pair + hh
                        nc.vector.tensor_copy(
                            v_s[:, tc_i, h * VW:h * VW + DH],
                            tp[:, hh * DH:(hh + 1) * DH])

        otn = qkv.tile([P, 2, T], BF16, tag="otn")
        ot65 = qkv.tile([VW, SL2 * H_LOC, IW], F32, tag="big",
                        name="ot65")

        # ---- main loop pieces ----
        pt_tiles = {}

        def _sexp_jc(sl2, pair, jc):
            """S^T matmuls + exp for one (i-slice, head-pair, j-chunk)."""
            i0 = sl2 * IW
            sp = [psum.tile([P, IW], F32, tag="ps",
                            name=f"sp{sl2}{pair}{jc}{hh}")
                  for hh in range(2)]
            for ib in range(2):
                for hh in range(2):
                    nc.tensor.matmul(
                        sp[hh][:, ib * 512:(ib + 1) * 512],
                        kt_s[hh * 64:(hh + 1) * 64, pair,
                             jc * P:(jc + 1) * P],
                        qt_s[hh * 64:(hh + 1) * 64, pair,
                             i0 + ib * 512:i0 + (ib + 1) * 512],
                        start=True, stop=True,
                        tile_position=(64 * hh, 0))
            for hh in range(2):
                ptt = ptpool.tile([P, IW], BF16, tag=f"pt{hh}",
                                  name=f"pt{hh}_{sl2}{pair}{jc}")
                pt_tiles[(sl2, pair, jc, hh)] = ptt
                nc.scalar.activation(
                    ptt[:], sp[hh][:],
                    AF.Exp, bias=cb_s[:, jc:jc + 1], scale=SCALE)

        def _pv_jc(sl2, pair, jc, ov):
            """PV chunk with 65th ones-column (denominator in row 64)."""
            for hh in range(2):
                h = 2 * pair + hh
                ptt = pt_tiles[(sl2, pair, jc, hh)]
                for ib in range(2):
                    nc.tensor.matmul(
                        ov[hh][:, ib * 512:(ib + 1) * 512],
                        v_s[:, jc, h * VW:(h + 1) * VW],
                        ptt[:, ib * 512:(ib + 1) * 512],
                        start=(jc == 0), stop=(jc == JCH - 1))

        def _ov_flush(sl2, pair, ov):
            for hh in range(2):
                k = sl2 * H_LOC + 2 * pair + hh
                nc.vector.tensor_copy(ot65[:, k, :], ov[hh][:])

        def _ov_alloc(sl2, pair, pool):
            return [pool.tile([VW, IW], F32,
                              tag="pv" if pool is pvps else "ps",
                              name=f"ov{sl2}{pair}{hh}") for hh in range(2)]

        # ---- emission: proj Q,K -> first S/exp overlaps proj V; each
        # later pair's exp stream interleaves the previous pair's PV at
        # j-chunk granularity (constant pt-slot pressure, no ACT stalls)
        _proj(xq_d, wq_s, qt_s, nc.sync, "xq", psum, nc.scalar.copy)
        _proj(xk_d, wk_s, kt_s, nc.scalar, "xk", psum, nc.scalar.copy)
        for jc in range(JCH):
            _sexp_jc(0, 0, jc)
        _proj(xv_d, wv_s, vt_s, nc.gpsimd, "xv", pvps, nc.vector.tensor_copy)
        _v_transpose()
        steps = [(0, 1), (1, 0), (1, 1)]
        prev = (0, 0)
        ov = _ov_alloc(*prev, pvps)
        for cur in steps:
            for jc in range(JCH):
                _sexp_jc(cur[0], cur[1], jc)
                _pv_jc(prev[0], prev[1], jc, ov)
            _ov_flush(*prev, ov)
            prev = cur
            # the last pair's PV accumulates in the ps pool (sp tiles are
            # dead by then) so the dance can claim pv slots while it drains
            ov = _ov_alloc(*prev, psum if cur == (1, 1) else pvps)
        for jc in range(JCH):
            _pv_jc(prev[0], prev[1], jc, ov)
        _ov_flush(*prev, ov)

        # ---- normalize + repack: O^T[65,i] -> O -> /denom -> O^T paired,
        # interleaved with the out projection per i-block ----
        def _dance(sl2, pair, ib):
            tp = pvps.tile([P, IW], F32, tag="pv",
                           name=f"dtp{sl2}{pair}{ib}")
            onorm = onpool.tile([P, P], BF16, tag="on")
            for hh in range(2):
                k = sl2 * H_LOC + 2 * pair + hh
                nc.tensor.transpose(
                    tp[:, hh * VW:(hh + 1) * VW],
                    ot65[:, k, ib * P:(ib + 1) * P],
                    identf[0:VW, 0:VW])
            for hh in range(2):
                rv = rvpool.tile([P, 1], F32, tag="rv",
                                 name=f"rv{sl2}{pair}{ib}{hh}")
                nc.vector.reciprocal(
                    rv[:], tp[:, hh * VW + DH:hh * VW + VW])
                nc.vector.tensor_scalar_mul(
                    onorm[:, hh * DH:(hh + 1) * DH],
                    tp[:, hh * VW:hh * VW + DH], rv[:])
            tpo = psum.tile([P, 1024], BF16, tag="ps",
                            name=f"dto{sl2}{pair}{ib}")
            nc.tensor.transpose(tpo[:, 0:P], onorm[:], identb[:])
            nc.scalar.copy(
                otn[:, pair, sl2 * IW + ib * P:sl2 * IW + (ib + 1) * P],
                tpo[:, 0:P])

        def _outproj(tb):
            ops = psum.tile([P, 1024], F32, tag="ps", name=f"op{tb}")
            for cc in range(2):
                lhsT = otn[:, cc, tb * P:(tb + 1) * P]
                for nb in range(2):
                    nc.tensor.matmul(
                        ops[:, nb * 512:(nb + 1) * 512], lhsT,
                        wo_s[:, cc, nb * 512:(nb + 1) * 512],
                        start=(cc == 0), stop=(cc == 1))
            ostage = opool.tile([P, D], BF16, tag="ostage")
            nc.scalar.copy(ostage[:], ops[:])
            nc.sync.dma_start(out_d[tb * P:(tb + 1) * P, :], ostage[:])

        for sl2 in range(SL2):
            for ib in range(IW // P):
                _dance(sl2, 0, ib)
                _dance(sl2, 1, ib)
                _outproj(sl2 * (IW // P) + ib)

    nc.compile()
    return nc


_last_results = None


def _host_cb(frac: np.ndarray, bs: float):
    """Per-j exp bias cb[p, jc] = scale*(c_j - maxc) - A, j = jc*128 + p,
    with c_j = -bs/f_j (raw logit units)."""
    cbs = []
    for b in range(B):
        f = np.maximum(frac[b].astype(np.float64), 1e-7)
        c = -bs / f
        cb = SCALE * (c - c.max()) - A_SHIFT
        cbs.append(np.ascontiguousarray(
            cb.reshape(JCH, P).T.astype(np.float32)))
    return cbs


def _prepare(inputs):
    """Build the program and per-core input maps from full inputs."""
    inp = {k: np.asarray(v) for k, v in inputs.items()}
    query, key, value = inp["query"], inp["key"], inp["value"]
    frac = inp["frac"]
    Wq, Wk, Wv, Wo = inp["Wq"], inp["Wk"], inp["Wv"], inp["Wo"]
    attn_bias = inp["attn_bias"]

    bs = float(np.sum(attn_bias.astype(np.float64)))
    cbs = _host_cb(np.asarray(frac, np.float32), bs)

    nc = _build_program()

    in_maps = []
    for c in range(N_CORES):
        b, g = c // H_LOC, c % H_LOC
        sl = slice(g * C_LOC, (g + 1) * C_LOC)
        in_maps.append({
            "xq": np.ascontiguousarray(query[b].T).astype(ml_dtypes.bfloat16),
            "xk": np.ascontiguousarray(key[b].T).astype(ml_dtypes.bfloat16),
            "xv": np.ascontiguousarray(value[b].T).astype(ml_dtypes.bfloat16),
            "wq": np.ascontiguousarray(Wq[sl, :].T).astype(ml_dtypes.bfloat16),
            "wk": np.ascontiguousarray(Wk[sl, :].T).astype(ml_dtypes.bfloat16),
            "wv": np.ascontiguousarray(Wv[sl, :].T).astype(ml_dtypes.bfloat16),
            "wo": np.ascontiguousarray(Wo[:, sl].T).astype(ml_dtypes.bfloat16),
            "cb": cbs[b],
        })
    return nc, in_maps


def kernel(**inputs) -> np.ndarray:
    nc, in_maps = _prepare(inputs)

    res = run_bass_kernel_spmd(nc, in_maps, list(range(N_CORES)))
    global _last_results
    _last_results = res

    out = np.zeros((B, T, D), dtype=np.float32)
    for c in range(N_CORES):
        out[c // H_LOC] += np.asarray(res.results[c]["out"]).astype(np.float32)
    out += np.asarray(inputs["bo"], dtype=np.float32)[None, None, :]
    return out
